# revision 29
# baseline (speedup 1.0000x reference)
"""Trainium2 Bass kernel for nn_CNLinkPredictor (gnn_message_passing), v2.

Data-parallel over target edges T (8192) across 8 NeuronCores (1024
edges/core).  v2 reworks the baseline around three findings from the HW
profile: fp32 matmuls run as 2 half-rate passes (4 cyc/row vs 1 for bf16),
ACT/DVE per-instruction overheads demand 512-wide ops, and ACT table
switches cost 1.3us.

Layout: channel-major [128 ch, 512 tok] supertiles (16 edges each);
 - all matmuls bf16 (PSUM f32); x is pre-cast to bf16 in DRAM.
 - LN has no affine (folded into following weights); stats are computed
   with per-token-column outputs via tok-chunk-stationary matmuls, the
   scalar math runs on [128,4] tiles, and rstd/-mu*rstd rows are
   transposed once and broadcast with rank-1 ones matmuls.
 - mean-subtraction for the q/k/v projections is folded into rank-1
   PSUM corrections (W@1 outer -mu*rstd), so z1 is just tok*rstd_bc.
 - per-key softmax bias exp(c_k) and key-validity are folded into a
   post-exp scaling eps of V (and the denominator aug channel); the
   cross-edge block mask is a constant bf16 0/1 multiply on E.
 - scores use a band-structured q ("qbds", built with SBUF->SBUF DMAs,
   zeros persistent) against dense channel-major k slices.
 - pooling = masked (valid/cnt) multiply + segmented DVE reduce.
 - phases grouped G=8 supertiles so ACT tables (sqrt/exp/gelu) load
   once per group.
"""

import sys
import threading

sys.path.insert(0, "/opt/trn_rl_repo")

import numpy as np
import ml_dtypes

import concourse.bass as bass
import concourse.bacc as bacc
import concourse.mybir as mybir
from concourse.tile import TileContext
from concourse.masks import make_identity
from concourse.bass_utils import run_bass_kernel_spmd

F32 = mybir.dt.float32
BF16 = mybir.dt.bfloat16
I32 = mybir.dt.int32
AF = mybir.ActivationFunctionType
ALU = mybir.AluOpType

N, C, H, O = 100000, 128, 256, 1
T, K = 8192, 32
NHEAD, DH, FF = 8, 16, 512
NCORES = 8
TC = T // NCORES          # 1024 edges per core
NST = TC // 16            # 64 supertiles (512 tokens / 16 edges each)
GRP = 8                   # supertiles per table-phase group
NEG = -1e9

bfa = lambda a: np.ascontiguousarray(np.asarray(a, np.float32)).astype(ml_dtypes.bfloat16)
f32a = lambda a: np.ascontiguousarray(np.asarray(a, np.float32))


def _build_nc(nst16, nst32):
    """nst16 K'=16 supertiles (32 edges each) then nst32 K'=32 (16 edges)."""
    nc = bacc.Bacc("TRN2", target_bir_lowering=False)
    tcn = 32 * nst16 + 16 * nst32      # edges per core (== TC)
    nst = nst16 + nst32                # total supertiles
    ne = TC // 128                     # 8 phase-A tiles (always full)

    dt = {}

    def din(name, shape, dtype=BF16):
        dt[name] = nc.dram_tensor(name, shape, dtype, kind="ExternalInput")
        return dt[name]

    # data
    din("xbf", [N, C])
    din("idx_cn", [128, 4 * (nst16 + nst32)], I32)
    din("idx_t0", [128, ne], I32)
    din("idx_t1", [128, ne], I32)
    din("negv", [128, 4 * (nst16 + nst32)], F32)
    din("msbig", [128, 512 * (nst16 + nst32)])
    # weights
    for nm in ["w0xT", "a1", "a2", "a3", "wqT", "wkT", "woutA", "woutB",
               "wf1T_0", "wf1T_1", "wf1T_2", "wf1T_3",
               "wf2T_0", "wf2T_1", "wf2T_2", "wf2T_3",
               "wx1_0", "wx1_1", "wxj1_0", "wxj1_1"]:
        din(nm, [128, 128])
    for ic in range(2):
        for oc in range(2):
            for nm in ["wx2", "wx3", "wxj2", "wl1"]:
                din(f"{nm}_{ic}{oc}", [128, 128])
    din("wv_aug", [128, 129])
    din("wl2_0", [128, 1])
    din("wl2_1", [128, 1])
    din("Bind32", [16, 512])
    din("Bind16", [32, 512])
    din("bandmask32", [128, 1024])
    din("bandmask16", [128, 1024])
    din("wmean", [128, 1])
    din("ones_rep", [128, 128])
    din("tokb_row", [1, 128])
    for nm in ["outb_col", "bff2_col", "eps_col", "epsd_col", "beta_col",
               "bx1_0", "bx1_1", "bx2_0", "bx2_1", "bx3_0", "bx3_1",
               "bxj1_0", "bxj1_1", "bxj2_0", "bxj2_1", "bl1_0", "bl1_1"]:
        din(nm, [128, 1], F32)
    for c4 in range(4):
        din(f"bff1_{c4}", [128, 1], F32)
    din("bl2", [1, 1], F32)

    ec_dram = nc.dram_tensor("ec_dram", [TC, 128], BF16)
    zrow_dram = nc.dram_tensor("zrow_dram", [nst16 + nst32, 2048], BF16)
    out_dram = nc.dram_tensor("out", [1, tcn], F32, kind="ExternalOutput")

    with TileContext(nc) as tc:
        with (
            nc.allow_low_precision(reason="bf16 pipeline validated vs ref"),
            tc.tile_pool(name="cpool", bufs=1) as cp,
            tc.tile_pool(name="wp", bufs=2) as wp,
            tc.tile_pool(name="mp", bufs=2) as mp,
            tc.tile_pool(name="p1024", bufs=2, space="PSUM") as p1024,
            tc.tile_pool(name="pacc", bufs=1, space="PSUM") as pacc,
            tc.tile_pool(name="pT", bufs=1, space="PSUM") as pTp,
            tc.tile_pool(name="pctx2", bufs=1, space="PSUM") as pctx2p,
        ):
            cs = {}
            for nm, t in dt.items():
                if nm in ("xbf", "msbig"):
                    continue
                tile = cp.tile(list(t.shape), t.dtype, tag=f"c_{nm}", name=nm)
                nc.sync.dma_start(tile[:], t[:])
                cs[nm] = tile

            ident = cp.tile([128, 128], BF16, tag="ident")
            make_identity(nc, ident[:])
            identf = cp.tile([128, 128], F32, tag="identf")
            make_identity(nc, identf[:])

            xijT_all = cp.tile([128, TC], BF16, tag="xijT_all")
            xcn_all = cp.tile([128, tcn], F32, tag="xcn_all")
            qbds = cp.tile([128, 4096], BF16, tag="qbds")
            nc.vector.memset(qbds[:], 0.0)

            # ---------------- PHASE A: per-edge EC + xijT ----------------
            for j in range(ne):
                xi = wp.tile([128, C], BF16, tag="xi")
                xj = wp.tile([128, C], BF16, tag="xj")
                nc.gpsimd.indirect_dma_start(
                    out=xi[:], out_offset=None, in_=dt["xbf"][:],
                    in_offset=bass.IndirectOffsetOnAxis(
                        ap=cs["idx_t0"][:, j:j + 1], axis=0))
                nc.gpsimd.indirect_dma_start(
                    out=xj[:], out_offset=None, in_=dt["xbf"][:],
                    in_offset=bass.IndirectOffsetOnAxis(
                        ap=cs["idx_t1"][:, j:j + 1], axis=0))
                xij = wp.tile([128, C], BF16, tag="xij")
                nc.vector.tensor_tensor(out=xij[:], in0=xi[:], in1=xj[:],
                                        op=ALU.mult)
                pt = pTp.tile([128, 512], BF16, tag="pT", name="pt")
                nc.tensor.transpose(pt[:, 0:128], xi[:], ident[:])
                nc.tensor.transpose(pt[:, 128:256], xj[:], ident[:])
                nc.tensor.transpose(pt[:, 256:384], xij[:], ident[:])
                xiT = wp.tile([128, 128], BF16, tag="xiT")
                nc.vector.tensor_copy(out=xiT[:], in_=pt[:, 0:128])
                xjT = wp.tile([128, 128], BF16, tag="xjT")
                nc.vector.tensor_copy(out=xjT[:], in_=pt[:, 128:256])
                nc.vector.tensor_copy(out=xijT_all[:, 128 * j:128 * (j + 1)],
                                      in_=pt[:, 256:384])

                ecp = pacc.tile([128, 512], F32, tag="pacc", name="ecp")
                nc.tensor.matmul(ecp[:, 0:128], lhsT=xiT[:], rhs=cs["a1"][:],
                                 start=True, stop=False)
                nc.tensor.matmul(ecp[:, 0:128], lhsT=xjT[:], rhs=cs["a2"][:],
                                 start=False, stop=False)
                nc.tensor.matmul(ecp[:, 0:128],
                                 lhsT=xijT_all[:, 128 * j:128 * (j + 1)],
                                 rhs=cs["a3"][:], start=False, stop=False)
                nc.tensor.matmul(ecp[:, 0:128], lhsT=cs["ones_rep"][0:1, :],
                                 rhs=cs["tokb_row"][:], start=False, stop=True)
                ec_s = wp.tile([128, 128], BF16, tag="ec_s")
                nc.vector.tensor_copy(out=ec_s[:], in_=ecp[:, 0:128])
                nc.sync.dma_start(ec_dram[128 * j:128 * (j + 1), :], ec_s[:])

            # ---------------- PHASE B: grouped supertiles ----------------
            SHUF16 = [16] * 32

            def _stats(t_a, t_b):
                """mean rows via 2 matmuls -> transpose -> [128,{mu4|ms2 4}]."""
                strows = p1024.tile([128, 1024], F32, tag="p1024",
                                    name="strows")
                nc.tensor.matmul(strows[0:1, 0:512], lhsT=cs["wmean"][:],
                                 rhs=t_a[:], start=True, stop=True)
                nc.tensor.matmul(strows[0:1, 512:1024], lhsT=cs["wmean"][:],
                                 rhs=t_b[:], start=True, stop=True)
                rows33 = wp.tile([33, 512], F32, tag="rows33", name="rows33")
                nc.vector.tensor_copy(out=rows33[0:1, :],
                                      in_=strows[0:1, 0:512])
                nc.vector.tensor_copy(out=rows33[32:33, :],
                                      in_=strows[0:1, 512:1024])
                stx = pacc.tile([128, 512], F32, tag="pacc", name="stx")
                for c in range(2):
                    nc.tensor.transpose(stx[:, 66 * c:66 * c + 33],
                                        rows33[:, 256 * c:256 * c + 128],
                                        identf[0:33, 0:33])
                    nc.tensor.transpose(stx[:, 66 * c + 33:66 * c + 66],
                                        rows33[:, 256 * c + 128:256 * (c + 1)],
                                        identf[0:33, 0:33])
                st_sb = wp.tile([128, 8], F32, tag="st_sb", name="st_sb",
                                bufs=9)
                sv = stx[:, 0:132].rearrange("p (c o) -> p c o", o=33)
                nc.vector.tensor_copy(
                    out=st_sb[:, 0:4].rearrange("p (c o) -> p c o", o=1),
                    in_=sv[:, :, 0:1])
                nc.vector.tensor_copy(
                    out=st_sb[:, 4:8].rearrange("p (c o) -> p c o", o=1),
                    in_=sv[:, :, 32:33])
                return st_sb

            def edge0_of(g):
                return 32 * g if g < nst16 else 32 * nst16 + 16 * (g - nst16)

            def ee_of(g):
                return 32 if g < nst16 else 16

            def s1(g):
                """gather + transpose + tok(relu) + LN1 raw stats."""
                xw = wp.tile([128, 512], BF16, tag="xw", name="xw")
                for s in range(4):
                    nc.gpsimd.indirect_dma_start(
                        out=xw[:, 128 * s:128 * (s + 1)], out_offset=None,
                        in_=dt["xbf"][:],
                        in_offset=bass.IndirectOffsetOnAxis(
                            ap=cs["idx_cn"][:, 4 * g + s:4 * g + s + 1],
                            axis=0))
                pt = pTp.tile([128, 512], BF16, tag="pT", name="pt")
                for s in range(4):
                    nc.tensor.transpose(pt[:, 128 * s:128 * (s + 1)],
                                        xw[:, 128 * s:128 * (s + 1)], ident[:])
                xwcm = wp.tile([128, 512], BF16, tag="xwcm", name="xwcm")
                nc.scalar.copy(xwcm[:], pt[:])
                e0, ee = edge0_of(g), ee_of(g)
                ecE = wp.tile([32, 128], BF16, tag="ecE", name="ecE")
                nc.sync.dma_start(ecE[0:ee, :], ec_dram[e0:e0 + ee, :])
                tokp = pacc.tile([128, 512], F32, tag="pacc", name="tokp")
                nc.tensor.matmul(tokp[:], lhsT=cs["w0xT"][:], rhs=xwcm[:],
                                 start=True, stop=False)
                bind = cs["Bind16"] if ee == 32 else cs["Bind32"]
                nc.tensor.matmul(tokp[:], lhsT=ecE[0:ee, :], rhs=bind[0:ee, :],
                                 start=False, stop=True)
                tok = wp.tile([128, 512], BF16, tag="tok", name="tok", bufs=9)
                nc.scalar.activation(tok[:], tokp[:], AF.Relu)
                sq = wp.tile([128, 512], BF16, tag="sq", name="sq")
                nc.vector.tensor_tensor(out=sq[:], in0=tok[:], in1=tok[:],
                                        op=ALU.mult)
                st_sb = _stats(tok, sq)
                return tok, st_sb

            def s2a(g, st_sb, tag):
                """[128,4] scalar math through ln(var+eps) (Ln table)."""
                mu = st_sb[:, 0:4]
                musq = wp.tile([128, 4], F32, tag="musq" + tag, name="musq")
                nc.vector.tensor_tensor(out=musq[:], in0=mu, in1=mu,
                                        op=ALU.mult)
                varr = wp.tile([128, 4], F32, tag="varr" + tag, name="varr")
                nc.vector.tensor_tensor(out=varr[:], in0=st_sb[:, 4:8],
                                        in1=musq[:], op=ALU.subtract)
                lnv = wp.tile([128, 4], F32, tag="lnv" + tag, name="lnv",
                              bufs=9)
                nc.scalar.activation(lnv[:], varr[:], AF.Ln,
                                     bias=cs["eps_col"][:, 0:1])
                return lnv

            def s2(g, st_sb, lnv, tag):
                """rstd=exp(-ln/2) (Exp table) -> broadcast rows via DRAM."""
                mu = st_sb[:, 0:4]
                rows = wp.tile([128, 8], F32, tag="rows" + tag, name="rows")
                nc.scalar.activation(rows[:, 0:4], lnv[:], AF.Exp,
                                     scale=-0.5)
                negmu = wp.tile([128, 4], F32, tag="negmu" + tag, name="negmu")
                nc.vector.tensor_scalar(out=negmu[:], in0=mu, scalar1=-1.0,
                                        scalar2=None, op0=ALU.mult)
                nc.vector.tensor_tensor(out=rows[:, 4:8], in0=negmu[:],
                                        in1=rows[:, 0:4], op=ALU.mult)
                rows_bf = wp.tile([128, 8], BF16, tag="rowsbf" + tag,
                                  name="rows_bf")
                nc.vector.tensor_copy(out=rows_bf[:], in_=rows[:])
                pt = pTp.tile([128, 512], BF16, tag="pT", name="pt")
                nc.tensor.transpose(pt[0:8, 0:128], rows_bf[:], ident[:])
                rsb = wp.tile([8, 128], BF16, tag="rsb" + tag, name="rsb")
                nc.vector.tensor_copy(out=rsb[:], in_=pt[0:8, 0:128])
                # stage through DRAM: zrow[g] = [rstd 512 | -mu*rstd 512]
                zoff = 0 if tag == "1" else 1024
                zr = zrow_dram[g:g + 1, zoff:zoff + 1024]
                dstw = bass.AP(tensor=zr.tensor, offset=zr.offset,
                               ap=[[2048, 1], [128, 8], [1, 128]])
                nc.sync.dma_start(dstw, rsb[:])
                zbc_sb = wp.tile([128, 1024], BF16, tag="zbc" + tag,
                                 name="zbc_sb", bufs=9)
                for j2 in range(2):
                    sap = zrow_dram[g:g + 1,
                                    zoff + 512 * j2:zoff + 512 * (j2 + 1)]
                    srcb = bass.AP(tensor=sap.tensor, offset=sap.offset,
                                   ap=[[0, 128], [1, 512]])
                    eng = nc.gpsimd if j2 == 0 else nc.sync
                    eng.dma_start(zbc_sb[:, 512 * j2:512 * (j2 + 1)], srcb)
                return zbc_sb

            def s3(g, tok, zbc_sb):
                """attention + out-proj + residual + LN2 raw stats."""
                z1t = wp.tile([128, 512], BF16, tag="z1t", name="z1t")
                nc.vector.tensor_tensor(out=z1t[:], in0=tok[:],
                                        in1=zbc_sb[:, 0:512], op=ALU.mult)
                z1 = wp.tile([128, 512], BF16, tag="z1", name="z1")
                nc.vector.tensor_tensor(out=z1[:], in0=z1t[:],
                                        in1=zbc_sb[:, 512:1024], op=ALU.add)
                qkp = p1024.tile([128, 1024], F32, tag="p1024", name="qkp")
                nc.tensor.matmul(qkp[:, 0:512], lhsT=cs["wqT"][:], rhs=z1[:],
                                 start=True, stop=True)
                nc.tensor.matmul(qkp[:, 512:1024], lhsT=cs["wkT"][:],
                                 rhs=z1[:], start=True, stop=True)
                qk = wp.tile([128, 1024], BF16, tag="qk", name="qk")
                nc.scalar.copy(qk[:, 0:512], qkp[:, 0:512])
                nc.scalar.copy(qk[:, 512:1024], qkp[:, 512:1024])
                # qbds bands via sbuf->sbuf DMA (zeros persistent)
                for h in range(8):
                    eng = nc.sync if h % 2 == 0 else nc.gpsimd
                    eng.dma_start(
                        qbds[16 * h:16 * h + 16, :].rearrange(
                            "p (s hh q) -> p s hh q", hh=8, q=128)[:, :, h, :],
                        qk[16 * h:16 * h + 16, 0:512].rearrange(
                            "p (s q) -> p s q", q=128))
                # v + eps + vaug per subtile
                vaugs = []
                vp = p1024.tile([128, 1024], F32, tag="p1024", name="vp")
                VOFF = [0, 129, 512, 641]
                for s in range(4):
                    nc.tensor.matmul(vp[:, VOFF[s]:VOFF[s] + 129],
                                     lhsT=z1[:, 128 * s:128 * (s + 1)],
                                     rhs=cs["wv_aug"][:],
                                     start=True, stop=True)
                vsb = []
                for p2 in range(2):
                    v2 = wp.tile([128, 258], BF16, tag="v_sb", name="v_sb",
                                 bufs=2)
                    nc.vector.tensor_copy(out=v2[:],
                                          in_=vp[:, 512 * p2:512 * p2 + 258])
                    vsb.append(v2)
                for s in range(4):
                    v_sb = vsb[s // 2][:, 129 * (s % 2):129 * (s % 2) + 129]
                    epsc = wp.tile([128, 1], F32, tag="epsc", name="epsc",
                                   bufs=4)
                    nc.scalar.activation(
                        epsc[:], v_sb[:, 128:129], AF.Exp,
                        bias=cs["negv"][:, 4 * g + s:4 * g + s + 1])
                    vaug = wp.tile([128, 256], BF16, tag="vaug",
                                   name="vaug", bufs=4)
                    vv = vaug[:].rearrange("p (h d) -> p h d", d=32)
                    nc.vector.tensor_scalar(
                        out=vv[:, :, 0:16],
                        in0=v_sb[:, 0:128].rearrange("p (h d) -> p h d",
                                                     d=16),
                        scalar1=epsc[:, 0:1], scalar2=None, op0=ALU.mult)
                    nc.gpsimd.tensor_copy(
                        out=vv[:, :, 16:17],
                        in_=epsc[:, 0:1].to_broadcast([128, 8, 1]))
                    vaugs.append(vaug)
                # scores -> exp -> mask -> ctx -> norm, per subtile
                ups = pacc.tile([128, 512], F32, tag="pacc", name="ups")
                cn_all = wp.tile([128, 1024], BF16, tag="cn_all",
                                 name="cn_all")
                for s in range(4):
                    scp = p1024.tile([128, 1024], F32, tag="p1024", name="scp")
                    for half in range(2):
                        nc.tensor.matmul(
                            scp[:, 512 * half:512 * (half + 1)],
                            lhsT=qk[:, 512 + 128 * s:512 + 128 * (s + 1)],
                            rhs=qbds[:, 1024 * s + 512 * half:
                                     1024 * s + 512 * (half + 1)],
                            start=True, stop=True)
                    E = wp.tile([128, 1024], BF16, tag="E", name="E")
                    nc.scalar.activation(E[:, 0:512], scp[:, 0:512], AF.Exp)
                    nc.scalar.activation(E[:, 512:1024], scp[:, 512:1024],
                                         AF.Exp)
                    Em = wp.tile([128, 1024], BF16, tag="Em", name="Em")
                    bm = (cs["bandmask16"] if ee_of(g) == 32
                          else cs["bandmask32"])
                    nc.vector.tensor_tensor(out=Em[:], in0=E[:],
                                            in1=bm[:], op=ALU.mult)
                    ctxp = pctx2p.tile([128, 1024], F32, tag="pctx2",
                                       name="ctxp")
                    for hf in range(2):
                        nc.tensor.matmul(
                            ctxp[:, 512 * hf:512 * (hf + 1)],
                            lhsT=vaugs[s][:, 128 * hf:128 * (hf + 1)],
                            rhs=Em[:, 512 * hf:512 * (hf + 1)],
                            start=True, stop=True)
                    cx = wp.tile([128, 256], F32, tag="cx", name="cx")
                    for hf in range(2):
                        for i2 in range(4):
                            nc.scalar.activation(
                                cx[32 * i2:32 * i2 + 17,
                                   128 * hf:128 * hf + 128],
                                ctxp[32 * i2:32 * i2 + 17,
                                     512 * hf + 128 * i2:
                                     512 * hf + 128 * (i2 + 1)],
                                AF.Identity,
                                bias=cs["epsd_col"][0:17, 0:1])
                    rt = wp.tile([128, 256], F32, tag="rt", name="rt")
                    nc.vector.stream_shuffle(rt[:], cx[:], SHUF16)
                    rtr = wp.tile([128, 256], F32, tag="rtr", name="rtr")
                    nc.vector.reciprocal_approx_fast(rtr[:], rt[:])
                    nc.vector.tensor_tensor(
                        out=cn_all[:, 256 * s:256 * (s + 1)], in0=cx[:],
                        in1=rtr[:], op=ALU.mult)
                cv = cn_all[:].rearrange("p (s2 hf q) -> p s2 hf q",
                                         s2=4, hf=2)
                nc.tensor.matmul(ups[:], lhsT=cs["woutA"][:],
                                 rhs=cv[:, :, 0, :], start=True, stop=False)
                nc.tensor.matmul(ups[:], lhsT=cs["woutB"][:],
                                 rhs=cv[:, :, 1, :], start=False, stop=True)
                tok2 = wp.tile([128, 512], BF16, tag="tok2", name="tok2",
                               bufs=9)
                nc.vector.scalar_tensor_tensor(
                    out=tok2[:], in0=ups[:], scalar=cs["outb_col"][:, 0:1],
                    in1=tok[:], op0=ALU.add, op1=ALU.add)
                sq2 = wp.tile([128, 512], BF16, tag="sq2", name="sq2")
                nc.vector.tensor_tensor(out=sq2[:], in0=tok2[:], in1=tok2[:],
                                        op=ALU.mult)
                st2_sb = _stats(tok2, sq2)
                return tok2, st2_sb

            def s5(g, tok2, zbc2_sb):
                """LN2 apply + FF + residual + masked pool -> xcn_all."""
                z2t = wp.tile([128, 512], BF16, tag="z2t", name="z2t")
                nc.vector.tensor_tensor(out=z2t[:], in0=tok2[:],
                                        in1=zbc2_sb[:, 0:512], op=ALU.mult)
                z2 = wp.tile([128, 512], BF16, tag="z2", name="z2")
                nc.vector.tensor_tensor(out=z2[:], in0=z2t[:],
                                        in1=zbc2_sb[:, 512:1024], op=ALU.add)
                gs = []
                for pair in range(2):
                    fp = p1024.tile([128, 1024], F32, tag="p1024", name="fp")
                    for i in range(2):
                        c4 = 2 * pair + i
                        nc.tensor.matmul(fp[:, 512 * i:512 * (i + 1)],
                                         lhsT=cs[f"wf1T_{c4}"][:], rhs=z2[:],
                                         start=True, stop=True)
                    for i in range(2):
                        c4 = 2 * pair + i
                        gt = wp.tile([128, 512], BF16, tag="gt", name="gt",
                                     bufs=4)
                        nc.scalar.activation(gt[:], fp[:, 512 * i:512 * (i + 1)],
                                             AF.Gelu,
                                             bias=cs[f"bff1_{c4}"][:, 0:1])
                        gs.append(gt)
                f2p = pacc.tile([128, 512], F32, tag="pacc", name="f2p")
                for c4 in range(4):
                    nc.tensor.matmul(f2p[:], lhsT=cs[f"wf2T_{c4}"][:],
                                     rhs=gs[c4][:], start=(c4 == 0),
                                     stop=(c4 == 3))
                tok3 = wp.tile([128, 512], BF16, tag="tok3", name="tok3")
                nc.vector.scalar_tensor_tensor(
                    out=tok3[:], in0=f2p[:], scalar=cs["bff2_col"][:, 0:1],
                    in1=tok2[:], op0=ALU.add, op1=ALU.add)
                msbc = wp.tile([128, 512], BF16, tag="msbc", name="msbc")
                nc.sync.dma_start(msbc[:],
                                  dt["msbig"][:, 512 * g:512 * (g + 1)])
                tok3m = wp.tile([128, 512], BF16, tag="tok3m", name="tok3m")
                nc.vector.tensor_tensor(out=tok3m[:], in0=tok3[:],
                                        in1=msbc[:], op=ALU.mult)
                e0, ee = edge0_of(g), ee_of(g)
                nc.vector.tensor_reduce(
                    out=xcn_all[:, e0:e0 + ee],
                    in_=tok3m[:].rearrange("p (e k) -> p e k", k=512 // ee),
                    axis=mybir.AxisListType.X, op=ALU.add)

            for g0 in range(0, nst, GRP):
                gs_ = list(range(g0, min(g0 + GRP, nst)))
                d1 = {g: s1(g) for g in gs_}
                l1 = {g: s2a(g, d1[g][1], "1") for g in gs_}
                r1 = {g: s2(g, d1[g][1], l1[g], "1") for g in gs_}
                d3 = {g: s3(g, d1[g][0], r1[g]) for g in gs_}
                l2 = {g: s2a(g, d3[g][1], "2") for g in gs_}
                r2 = {g: s2(g, d3[g][1], l2[g], "2") for g in gs_}
                for g in gs_:
                    s5(g, d3[g][0], r2[g])

            # ---------------- PHASE C: edge MLPs -------------------------
            xcn_bf = cp.tile([128, tcn], BF16, tag="xcn_bf")
            nc.vector.tensor_copy(out=xcn_bf[:], in_=xcn_all[:])

            def dense(rhs_tiles, win, bin_, act, n_ic, out_tag, w):
                outs = []
                for oc in range(2):
                    o = mp.tile([128, w], BF16, tag=f"{out_tag}{oc}",
                                name=out_tag)
                    for nh in range(max(1, w // 512)):
                        cw = min(512, w)
                        p5 = p1024.tile([128, 1024], F32, tag="p1024",
                                        name="p5")
                        for ic in range(n_ic):
                            wt = cs[win(ic, oc)]
                            r = (rhs_tiles if n_ic == 1 else rhs_tiles[ic])
                            nc.tensor.matmul(
                                p5[:, :cw], lhsT=wt[:],
                                rhs=r[:, cw * nh:cw * (nh + 1)],
                                start=(ic == 0), stop=(ic == n_ic - 1))
                        nc.scalar.activation(
                            o[:, cw * nh:cw * (nh + 1)], p5[:, :cw], act,
                            bias=cs[bin_(oc)][:, 0:1])
                    outs.append(o)
                return outs

            def _phase_c(lo, w):
                h1 = dense(xcn_bf[:, lo:lo + w], lambda ic, oc: f"wx1_{oc}",
                           lambda oc: f"bx1_{oc}", AF.Relu, 1, "h1_", w)
                h2 = dense(h1, lambda ic, oc: f"wx2_{ic}{oc}",
                           lambda oc: f"bx2_{oc}", AF.Relu, 2, "h2_", w)
                h3 = dense(h2, lambda ic, oc: f"wx3_{ic}{oc}",
                           lambda oc: f"bx3_{oc}", AF.Identity, 2, "h3_", w)
                j1 = dense(xijT_all[:, lo:lo + w],
                           lambda ic, oc: f"wxj1_{oc}",
                           lambda oc: f"bxj1_{oc}", AF.Relu, 1, "j1_", w)
                j2 = dense(j1, lambda ic, oc: f"wxj2_{ic}{oc}",
                           lambda oc: f"bxj2_{oc}", AF.Identity, 2, "j2_", w)
                zi = []
                for oc in range(2):
                    z = mp.tile([128, w], BF16, tag=f"zi{oc}", name="zi")
                    nc.vector.scalar_tensor_tensor(
                        out=z[:], in0=h3[oc][:], scalar=cs["beta_col"][:, 0:1],
                        in1=j2[oc][:], op0=ALU.mult, op1=ALU.add)
                    zi.append(z)
                zz = dense(zi, lambda ic, oc: f"wl1_{ic}{oc}",
                           lambda oc: f"bl1_{oc}", AF.Relu, 2, "zz", w)
                osb = mp.tile([1, w], BF16, tag="osb", name="osb")
                cw = min(512, w)
                for nh in range(max(1, w // 512)):
                    fo = pacc.tile([128, 512], F32, tag="pacc", name="fo")
                    nc.tensor.matmul(fo[0:1, :cw], lhsT=cs["wl2_0"][:],
                                     rhs=zz[0][:, cw * nh:cw * (nh + 1)],
                                     start=True, stop=False)
                    nc.tensor.matmul(fo[0:1, :cw], lhsT=cs["wl2_1"][:],
                                     rhs=zz[1][:, cw * nh:cw * (nh + 1)],
                                     start=False, stop=True)
                    nc.scalar.activation(osb[0:1, cw * nh:cw * (nh + 1)],
                                         fo[0:1, :cw], AF.Identity,
                                         bias=cs["bl2"][0:1, 0:1])
                osf = mp.tile([1, w], F32, tag="osf", name="osf")
                nc.vector.tensor_copy(out=osf[:], in_=osb[:])
                nc.sync.dma_start(out_dram[0:1, lo:lo + w], osf[:])

            for lo in range(0, tcn, 512):
                _phase_c(lo, min(512, tcn - lo))

    nc.finalize()
    return nc


# ---------------------------------------------------------------- host side

def _prep_shared(inp):
    f = lambda k: np.asarray(inp[k], np.float32)
    tok_w, tok_b = f("tok_w"), f("tok_b")
    g1, b1 = f("ln1_g"), f("ln1_b")
    qkv_w, qkv_b = f("qkv_w"), f("qkv_b")
    out_w, out_b = f("out_w"), f("out_b")
    g2, b2 = f("ln2_g"), f("ln2_b")
    ff1_w, ff1_b = f("ff1_w"), f("ff1_b")
    ff2_w, ff2_b = f("ff2_w"), f("ff2_b")

    d = {}
    d["w0xT"] = bfa(tok_w[:, :C].T)
    d["a1"] = bfa(tok_w[:, C:2 * C].T)
    d["a2"] = bfa(tok_w[:, 2 * C:3 * C].T)
    d["a3"] = bfa(tok_w[:, 3 * C:4 * C].T)
    d["tokb_row"] = bfa(tok_b[None, :])

    sc = 1.0 / np.sqrt(DH)
    Wq, Wk, Wv = qkv_w[:C], qkv_w[C:2 * C], qkv_w[2 * C:3 * C]
    bq, bk, bv = qkv_b[:C], qkv_b[C:2 * C], qkv_b[2 * C:3 * C]
    Wq_e = Wq * g1[None, :] * sc
    bq_e = (Wq @ b1) * sc + bq * sc
    Wk_e = Wk * g1[None, :]
    Wv_e = Wv * g1[None, :]
    bv_e = Wv @ b1 + bv
    w_ck = Wk_e.T @ bq_e
    ones = np.ones(C, np.float32)
    d["wqT"] = bfa(Wq_e.T)
    d["wkT"] = bfa(Wk_e.T)
    d["wv_aug"] = bfa(np.concatenate([Wv_e.T, w_ck[:, None]], axis=1))

    for nm, heads in (("woutA", [0, 1, 2, 3]), ("woutB", [4, 5, 6, 7])):
        w = np.zeros((128, 128), np.float32)
        for i, h in enumerate(heads):
            w[32 * i:32 * i + 16, :] = out_w[:, 16 * h:16 * h + 16].T
        d[nm] = bfa(w)
    d["outb_col"] = f32a((out_b + out_w @ bv_e)[:, None])

    for c4 in range(4):
        sl = slice(128 * c4, 128 * (c4 + 1))
        d[f"wf1T_{c4}"] = bfa((ff1_w[sl, :] * g2[None, :]).T)
        d[f"bff1_{c4}"] = f32a((ff1_w[sl, :] @ b2 + ff1_b[sl])[:, None])
        d[f"wf2T_{c4}"] = bfa(ff2_w[:, sl].T)
    d["bff2_col"] = f32a(ff2_b[:, None])

    for nm, wkey, bkey in (("wx1", "xcn_w1", "xcn_b1"),
                           ("wxj1", "xij_w1", "xij_b1")):
        W, B = f(wkey), f(bkey)
        for oc in range(2):
            sl = slice(128 * oc, 128 * (oc + 1))
            d[f"{nm}_{oc}"] = bfa(W[sl, :].T)
            d[f"b{nm[1:]}_{oc}"] = f32a(B[sl][:, None])
    for nm, wkey, bkey in (("wx2", "xcn_w2", "xcn_b2"),
                           ("wx3", "xcn_w3", "xcn_b3"),
                           ("wxj2", "xij_w2", "xij_b2"),
                           ("wl1", "lin_w1", "lin_b1")):
        W, B = f(wkey), f(bkey)
        for ic in range(2):
            for oc in range(2):
                d[f"{nm}_{ic}{oc}"] = bfa(
                    W[128 * oc:128 * (oc + 1), 128 * ic:128 * (ic + 1)].T)
        for oc in range(2):
            d[f"b{nm[1:]}_{oc}"] = f32a(B[128 * oc:128 * (oc + 1)][:, None])
    lin_w2, lin_b2 = f("lin_w2"), f("lin_b2")
    d["wl2_0"] = bfa(lin_w2[0, :128][:, None])
    d["wl2_1"] = bfa(lin_w2[0, 128:][:, None])
    d["bl2"] = f32a(lin_b2.reshape(1, 1))

    for kk in (16, 32):
        ee = 512 // kk
        Bind = np.zeros((ee, 512), np.float32)
        for e in range(ee):
            Bind[e, kk * e:kk * (e + 1)] = 1.0
        d[f"Bind{kk}"] = bfa(Bind)
        bm = np.zeros((128, 1024), np.float32)
        epb = 128 // kk          # edges per 128-token subtile
        for h in range(8):
            for e in range(epb):
                bm[kk * e:kk * (e + 1),
                   128 * h + kk * e:128 * h + kk * (e + 1)] = 1.0
        d[f"bandmask{kk}"] = bfa(bm)
    d["wmean"] = bfa(np.full((128, 1), 1.0 / 128.0))
    d["ones_rep"] = bfa(np.ones((128, 128)))
    d["eps_col"] = f32a(np.full((128, 1), 1e-5))
    d["epsd_col"] = f32a(np.full((128, 1), 1e-30))
    d["beta_col"] = f32a(np.full((128, 1),
                                 np.asarray(inp["beta"],
                                            np.float32).reshape(-1)[0]))
    return d


def _prep_core(inp, core, nst16, nst32, perm):
    sl = slice(core * TC, (core + 1) * TC)
    tar = np.asarray(inp["tar_ei"])[:, sl].astype(np.int32)[:, perm]
    cols = np.asarray(inp["cn_cols"])[sl].astype(np.int32)[perm]   # [TC, K]
    cnt = np.asarray(inp["cn_counts"])[sl].astype(np.int64)[perm]  # [TC]

    nst = nst16 + nst32
    ntok = 512 * nst
    # token -> (edge, k) maps for the two buckets
    t16 = np.arange(512 * nst16)
    e16, k16 = t16 // 16, t16 % 16
    t32 = np.arange(512 * nst32)
    e32, k32 = 32 * nst16 + t32 // 32, t32 % 32
    e_arr = np.concatenate([e16, e32])
    k_arr = np.concatenate([k16, k32])

    d = {}
    nsub = ntok // 128
    idx_flat = cols[e_arr, k_arr]
    d["idx_cn"] = np.ascontiguousarray(idx_flat.reshape(nsub, 128).T)
    ne = TC // 128
    d["idx_t0"] = np.ascontiguousarray(tar[0].reshape(ne, 128).T)
    d["idx_t1"] = np.ascontiguousarray(tar[1].reshape(ne, 128).T)

    valid = (k_arr < cnt[e_arr])
    vcol = np.ascontiguousarray(valid.reshape(nsub, 128).T)
    d["negv"] = ((~vcol) * np.float32(NEG)).astype(np.float32)

    ms = valid.astype(np.float32) / np.maximum(cnt[e_arr], 1).astype(np.float32)
    d["msbig"] = np.ascontiguousarray(
        np.broadcast_to(bfa(ms[None, :]), (128, ntok)))
    return d


def _split_counts(inp):
    """Global (nst16, nst32) and per-core permutations."""
    cnts = np.asarray(inp["cn_counts"]).reshape(NCORES, TC)
    n16 = (cnts <= 16).sum(axis=1)
    nst16 = int(min(n16) // 32)
    nst32 = (TC - 32 * nst16) // 16
    perms = [np.argsort(cnts[c], kind="stable") for c in range(NCORES)]
    return nst16, nst32, perms


_CACHE = {}
_CACHE_LOCK = threading.Lock()


def _get_nc(key):
    with _CACHE_LOCK:
        if key not in _CACHE:
            _CACHE[key] = _build_nc(*key)
        return _CACHE[key]


def run(inputs, nst=None, **spmd_kwargs):
    nst16, nst32, perms = _split_counts(inputs)
    nc = _get_nc((nst16, nst32))
    shared = _prep_shared(inputs)
    xbf = np.ascontiguousarray(
        np.asarray(inputs["x"], np.float32)).astype(ml_dtypes.bfloat16)
    in_maps = []
    for core in range(NCORES):
        m = dict(shared)
        m["xbf"] = xbf
        m.update(_prep_core(inputs, core, nst16, nst32, perms[core]))
        in_maps.append(m)
    res = run_bass_kernel_spmd(nc, in_maps, core_ids=list(range(NCORES)),
                               **spmd_kwargs)
    out = np.zeros((NCORES, TC), np.float32)
    for c in range(NCORES):
        out[c, perms[c]] = res.results[c]["out"][0]
    return out, res


def kernel(**inputs):
    out, _ = run(inputs)
    return out.reshape(T, O).astype(np.float32)


# revision 30
# speedup vs baseline: 1.3225x; 1.3225x over previous
"""Trainium2 Bass kernel for nn_CNLinkPredictor (gnn_message_passing), v2.

Data-parallel over target edges T (8192) across 8 NeuronCores (1024
edges/core).  v2 reworks the baseline around three findings from the HW
profile: fp32 matmuls run as 2 half-rate passes (4 cyc/row vs 1 for bf16),
ACT/DVE per-instruction overheads demand 512-wide ops, and ACT table
switches cost 1.3us.

Layout: channel-major [128 ch, 512 tok] supertiles (16 edges each);
 - all matmuls bf16 (PSUM f32); x is pre-cast to bf16 in DRAM.
 - LN has no affine (folded into following weights); stats are computed
   with per-token-column outputs via tok-chunk-stationary matmuls, the
   scalar math runs on [128,4] tiles, and rstd/-mu*rstd rows are
   transposed once and broadcast with rank-1 ones matmuls.
 - mean-subtraction for the q/k/v projections is folded into rank-1
   PSUM corrections (W@1 outer -mu*rstd), so z1 is just tok*rstd_bc.
 - per-key softmax bias exp(c_k) and key-validity are folded into a
   post-exp scaling eps of V (and the denominator aug channel); the
   cross-edge block mask is a constant bf16 0/1 multiply on E.
 - scores use a band-structured q ("qbds", built with SBUF->SBUF DMAs,
   zeros persistent) against dense channel-major k slices.
 - pooling = masked (valid/cnt) multiply + segmented DVE reduce.
 - phases grouped G=8 supertiles so ACT tables (sqrt/exp/gelu) load
   once per group.
"""

import sys
import threading

sys.path.insert(0, "/opt/trn_rl_repo")

import numpy as np
import ml_dtypes

import concourse.bass as bass
import concourse.bacc as bacc
import concourse.mybir as mybir
from concourse.tile import TileContext
from concourse.masks import make_identity
from concourse.bass_utils import run_bass_kernel_spmd

F32 = mybir.dt.float32
BF16 = mybir.dt.bfloat16
I32 = mybir.dt.int32
AF = mybir.ActivationFunctionType
ALU = mybir.AluOpType

N, C, H, O = 100000, 128, 256, 1
T, K = 8192, 32
NHEAD, DH, FF = 8, 16, 512
NCORES = 8
TC = T // NCORES          # 1024 edges per core
NST = TC // 16            # 64 supertiles (512 tokens / 16 edges each)
GRP = 8                   # supertiles per table-phase group
NEG = -1e9

bfa = lambda a: np.ascontiguousarray(np.asarray(a, np.float32)).astype(ml_dtypes.bfloat16)
f32a = lambda a: np.ascontiguousarray(np.asarray(a, np.float32))


def _build_nc(nst16, nst32):
    """nst16 K'=16 supertiles (32 edges each) then nst32 K'=32 (16 edges)."""
    nc = bacc.Bacc("TRN2", target_bir_lowering=False)
    tcn = 32 * nst16 + 16 * nst32      # edges per core (== TC)
    nst = nst16 + nst32                # total supertiles
    ne = TC // 128                     # 8 phase-A tiles (always full)

    dt = {}

    def din(name, shape, dtype=BF16):
        dt[name] = nc.dram_tensor(name, shape, dtype, kind="ExternalInput")
        return dt[name]

    # data
    din("xbf", [N, C])
    din("idx_cn", [128, 4 * (nst16 + nst32)], I32)
    din("idx_t0", [128, ne], I32)
    din("idx_t1", [128, ne], I32)
    din("negv", [128, 4 * (nst16 + nst32)], F32)
    din("msbig", [128, 512 * (nst16 + nst32)])
    # weights
    for nm in ["w0xT", "a1", "a2", "a3", "wqT", "wkT", "woutA", "woutB",
               "wf1T_0", "wf1T_1", "wf1T_2", "wf1T_3",
               "wf2T_0", "wf2T_1", "wf2T_2", "wf2T_3",
               "wx1_0", "wx1_1", "wxj1_0", "wxj1_1"]:
        din(nm, [128, 128])
    for ic in range(2):
        for oc in range(2):
            for nm in ["wx2", "wx3", "wxj2", "wl1"]:
                din(f"{nm}_{ic}{oc}", [128, 128])
    din("wv_aug", [128, 129])
    din("wl2_0", [128, 1])
    din("wl2_1", [128, 1])
    din("Bind32", [16, 512])
    din("Bind16", [32, 512])
    din("bandmask32", [128, 1024])
    din("bandmask16", [128, 1024])
    din("wmean", [128, 1])
    din("ones_rep", [128, 128])
    din("tokb_row", [1, 128])
    for nm in ["outb_col", "bff2_col", "eps_col", "epsd_col", "beta_col",
               "bx1_0", "bx1_1", "bx2_0", "bx2_1", "bx3_0", "bx3_1",
               "bxj1_0", "bxj1_1", "bxj2_0", "bxj2_1", "bl1_0", "bl1_1"]:
        din(nm, [128, 1], F32)
    for c4 in range(4):
        din(f"bff1_{c4}", [128, 1], F32)
    din("bl2", [1, 1], F32)

    ec_dram = nc.dram_tensor("ec_dram", [TC, 128], BF16)
    zrow_dram = nc.dram_tensor("zrow_dram", [nst16 + nst32, 2048], BF16)
    out_dram = nc.dram_tensor("out", [1, tcn], F32, kind="ExternalOutput")

    with TileContext(nc) as tc:
        with (
            nc.allow_low_precision(reason="bf16 pipeline validated vs ref"),
            tc.tile_pool(name="cpool", bufs=1) as cp,
            tc.tile_pool(name="wp", bufs=2) as wp,
            tc.tile_pool(name="mp", bufs=2) as mp,
            tc.tile_pool(name="p1024", bufs=2, space="PSUM") as p1024,
            tc.tile_pool(name="pacc", bufs=1, space="PSUM") as pacc,
            tc.tile_pool(name="pT", bufs=1, space="PSUM") as pTp,
            tc.tile_pool(name="pctx2", bufs=1, space="PSUM") as pctx2p,
        ):
            cs = {}
            for nm, t in dt.items():
                if nm in ("xbf", "msbig"):
                    continue
                tile = cp.tile(list(t.shape), t.dtype, tag=f"c_{nm}", name=nm)
                nc.sync.dma_start(tile[:], t[:])
                cs[nm] = tile

            ident = cp.tile([128, 128], BF16, tag="ident")
            make_identity(nc, ident[:])
            identf = cp.tile([128, 128], F32, tag="identf")
            make_identity(nc, identf[:])

            xijT_all = cp.tile([128, TC], BF16, tag="xijT_all")
            xcn_all = cp.tile([128, tcn], F32, tag="xcn_all")
            qbds = cp.tile([128, 4096], BF16, tag="qbds")
            nc.vector.memset(qbds[:], 0.0)

            # ---------------- PHASE A: per-edge EC + xijT ----------------
            for j in range(ne):
                xi = wp.tile([128, C], BF16, tag="xi")
                xj = wp.tile([128, C], BF16, tag="xj")
                nc.gpsimd.indirect_dma_start(
                    out=xi[:], out_offset=None, in_=dt["xbf"][:],
                    in_offset=bass.IndirectOffsetOnAxis(
                        ap=cs["idx_t0"][:, j:j + 1], axis=0))
                nc.gpsimd.indirect_dma_start(
                    out=xj[:], out_offset=None, in_=dt["xbf"][:],
                    in_offset=bass.IndirectOffsetOnAxis(
                        ap=cs["idx_t1"][:, j:j + 1], axis=0))
                xij = wp.tile([128, C], BF16, tag="xij")
                nc.vector.tensor_tensor(out=xij[:], in0=xi[:], in1=xj[:],
                                        op=ALU.mult)
                pt = pTp.tile([128, 512], BF16, tag="pT", name="pt")
                nc.tensor.transpose(pt[:, 0:128], xi[:], ident[:])
                nc.tensor.transpose(pt[:, 128:256], xj[:], ident[:])
                nc.tensor.transpose(pt[:, 256:384], xij[:], ident[:])
                xiT = wp.tile([128, 128], BF16, tag="xiT")
                nc.vector.tensor_copy(out=xiT[:], in_=pt[:, 0:128])
                xjT = wp.tile([128, 128], BF16, tag="xjT")
                nc.vector.tensor_copy(out=xjT[:], in_=pt[:, 128:256])
                nc.vector.tensor_copy(out=xijT_all[:, 128 * j:128 * (j + 1)],
                                      in_=pt[:, 256:384])

                ecp = pacc.tile([128, 512], F32, tag="pacc", name="ecp")
                nc.tensor.matmul(ecp[:, 0:128], lhsT=xiT[:], rhs=cs["a1"][:],
                                 start=True, stop=False)
                nc.tensor.matmul(ecp[:, 0:128], lhsT=xjT[:], rhs=cs["a2"][:],
                                 start=False, stop=False)
                nc.tensor.matmul(ecp[:, 0:128],
                                 lhsT=xijT_all[:, 128 * j:128 * (j + 1)],
                                 rhs=cs["a3"][:], start=False, stop=False)
                nc.tensor.matmul(ecp[:, 0:128], lhsT=cs["ones_rep"][0:1, :],
                                 rhs=cs["tokb_row"][:], start=False, stop=True)
                ec_s = wp.tile([128, 128], BF16, tag="ec_s")
                nc.vector.tensor_copy(out=ec_s[:], in_=ecp[:, 0:128])
                nc.sync.dma_start(ec_dram[128 * j:128 * (j + 1), :], ec_s[:])

            # ---------------- PHASE B: grouped supertiles ----------------
            SHUF16 = [16] * 32

            def _stats(t_a, t_b):
                """mean rows via 2 matmuls -> transpose -> [128,{mu4|ms2 4}]."""
                strows = p1024.tile([128, 1024], F32, tag="p1024",
                                    name="strows")
                nc.tensor.matmul(strows[0:1, 0:512], lhsT=cs["wmean"][:],
                                 rhs=t_a[:], start=True, stop=True)
                nc.tensor.matmul(strows[0:1, 512:1024], lhsT=cs["wmean"][:],
                                 rhs=t_b[:], start=True, stop=True)
                rows33 = wp.tile([33, 512], F32, tag="rows33", name="rows33")
                nc.vector.tensor_copy(out=rows33[0:1, :],
                                      in_=strows[0:1, 0:512])
                nc.vector.tensor_copy(out=rows33[32:33, :],
                                      in_=strows[0:1, 512:1024])
                stx = pacc.tile([128, 512], F32, tag="pacc", name="stx")
                for c in range(2):
                    nc.tensor.transpose(stx[:, 66 * c:66 * c + 33],
                                        rows33[:, 256 * c:256 * c + 128],
                                        identf[0:33, 0:33])
                    nc.tensor.transpose(stx[:, 66 * c + 33:66 * c + 66],
                                        rows33[:, 256 * c + 128:256 * (c + 1)],
                                        identf[0:33, 0:33])
                st_sb = wp.tile([128, 8], F32, tag="st_sb", name="st_sb",
                                bufs=9)
                sv = stx[:, 0:132].rearrange("p (c o) -> p c o", o=33)
                nc.vector.tensor_copy(
                    out=st_sb[:, 0:4].rearrange("p (c o) -> p c o", o=1),
                    in_=sv[:, :, 0:1])
                nc.vector.tensor_copy(
                    out=st_sb[:, 4:8].rearrange("p (c o) -> p c o", o=1),
                    in_=sv[:, :, 32:33])
                return st_sb

            def edge0_of(g):
                return 32 * g if g < nst16 else 32 * nst16 + 16 * (g - nst16)

            def ee_of(g):
                return 32 if g < nst16 else 16

            def s1(g):
                """gather + transpose + tok(relu) + LN1 raw stats."""
                xw = wp.tile([128, 512], BF16, tag="xw", name="xw")
                for s in range(4):
                    nc.gpsimd.indirect_dma_start(
                        out=xw[:, 128 * s:128 * (s + 1)], out_offset=None,
                        in_=dt["xbf"][:],
                        in_offset=bass.IndirectOffsetOnAxis(
                            ap=cs["idx_cn"][:, 4 * g + s:4 * g + s + 1],
                            axis=0))
                pt = pTp.tile([128, 512], BF16, tag="pT", name="pt")
                for s in range(4):
                    nc.tensor.transpose(pt[:, 128 * s:128 * (s + 1)],
                                        xw[:, 128 * s:128 * (s + 1)], ident[:])
                xwcm = wp.tile([128, 512], BF16, tag="xwcm", name="xwcm")
                nc.scalar.copy(xwcm[:], pt[:])
                e0, ee = edge0_of(g), ee_of(g)
                ecE = wp.tile([32, 128], BF16, tag="ecE", name="ecE")
                nc.sync.dma_start(ecE[0:ee, :], ec_dram[e0:e0 + ee, :])
                tokp = pacc.tile([128, 512], F32, tag="pacc", name="tokp")
                nc.tensor.matmul(tokp[:], lhsT=cs["w0xT"][:], rhs=xwcm[:],
                                 start=True, stop=False)
                bind = cs["Bind16"] if ee == 32 else cs["Bind32"]
                nc.tensor.matmul(tokp[:], lhsT=ecE[0:ee, :], rhs=bind[0:ee, :],
                                 start=False, stop=True)
                tok = wp.tile([128, 512], BF16, tag="tok", name="tok", bufs=9)
                nc.scalar.activation(tok[:], tokp[:], AF.Relu)
                sq = wp.tile([128, 512], BF16, tag="sq", name="sq")
                nc.vector.tensor_tensor(out=sq[:], in0=tok[:], in1=tok[:],
                                        op=ALU.mult)
                st_sb = _stats(tok, sq)
                return tok, st_sb

            def s2a(g, st_sb, tag):
                """[128,4] scalar math through ln(var+eps) (Ln table)."""
                mu = st_sb[:, 0:4]
                musq = wp.tile([128, 4], F32, tag="musq" + tag, name="musq")
                nc.vector.tensor_tensor(out=musq[:], in0=mu, in1=mu,
                                        op=ALU.mult)
                varr = wp.tile([128, 4], F32, tag="varr" + tag, name="varr")
                nc.vector.tensor_tensor(out=varr[:], in0=st_sb[:, 4:8],
                                        in1=musq[:], op=ALU.subtract)
                lnv = wp.tile([128, 4], F32, tag="lnv" + tag, name="lnv",
                              bufs=9)
                nc.scalar.activation(lnv[:], varr[:], AF.Ln,
                                     bias=cs["eps_col"][:, 0:1])
                return lnv

            def s2(g, st_sb, lnv, tag):
                """rstd=exp(-ln/2) (Exp table) -> broadcast rows via DRAM."""
                mu = st_sb[:, 0:4]
                rows = wp.tile([128, 8], F32, tag="rows" + tag, name="rows")
                nc.scalar.activation(rows[:, 0:4], lnv[:], AF.Exp,
                                     scale=-0.5)
                negmu = wp.tile([128, 4], F32, tag="negmu" + tag, name="negmu")
                nc.vector.tensor_scalar(out=negmu[:], in0=mu, scalar1=-1.0,
                                        scalar2=None, op0=ALU.mult)
                nc.vector.tensor_tensor(out=rows[:, 4:8], in0=negmu[:],
                                        in1=rows[:, 0:4], op=ALU.mult)
                rows_bf = wp.tile([128, 8], BF16, tag="rowsbf" + tag,
                                  name="rows_bf")
                nc.vector.tensor_copy(out=rows_bf[:], in_=rows[:])
                pt = pTp.tile([128, 512], BF16, tag="pT", name="pt")
                nc.tensor.transpose(pt[0:8, 0:128], rows_bf[:], ident[:])
                rsb = wp.tile([8, 128], BF16, tag="rsb" + tag, name="rsb")
                nc.vector.tensor_copy(out=rsb[:], in_=pt[0:8, 0:128])
                # stage through DRAM: zrow[g] = [rstd 512 | -mu*rstd 512]
                zoff = 0 if tag == "1" else 1024
                zr = zrow_dram[g:g + 1, zoff:zoff + 1024]
                dstw = bass.AP(tensor=zr.tensor, offset=zr.offset,
                               ap=[[2048, 1], [128, 8], [1, 128]])
                nc.sync.dma_start(dstw, rsb[:])
                zbc_sb = wp.tile([128, 1024], BF16, tag="zbc" + tag,
                                 name="zbc_sb", bufs=9)
                for j2 in range(2):
                    sap = zrow_dram[g:g + 1,
                                    zoff + 512 * j2:zoff + 512 * (j2 + 1)]
                    srcb = bass.AP(tensor=sap.tensor, offset=sap.offset,
                                   ap=[[0, 128], [1, 512]])
                    eng = nc.gpsimd if j2 == 0 else nc.sync
                    eng.dma_start(zbc_sb[:, 512 * j2:512 * (j2 + 1)], srcb)
                return zbc_sb

            def s3(g, tok, zbc_sb):
                """attention + out-proj + residual + LN2 raw stats."""
                z1t = wp.tile([128, 512], BF16, tag="z1t", name="z1t")
                nc.vector.tensor_tensor(out=z1t[:], in0=tok[:],
                                        in1=zbc_sb[:, 0:512], op=ALU.mult)
                z1 = wp.tile([128, 512], BF16, tag="z1", name="z1")
                nc.vector.tensor_tensor(out=z1[:], in0=z1t[:],
                                        in1=zbc_sb[:, 512:1024], op=ALU.add)
                qkp = p1024.tile([128, 1024], F32, tag="p1024", name="qkp")
                nc.tensor.matmul(qkp[:, 0:512], lhsT=cs["wqT"][:], rhs=z1[:],
                                 start=True, stop=True)
                nc.tensor.matmul(qkp[:, 512:1024], lhsT=cs["wkT"][:],
                                 rhs=z1[:], start=True, stop=True)
                qk = wp.tile([128, 1024], BF16, tag="qk", name="qk")
                nc.scalar.copy(qk[:, 0:512], qkp[:, 0:512])
                nc.scalar.copy(qk[:, 512:1024], qkp[:, 512:1024])
                # qbds bands via sbuf->sbuf DMA (zeros persistent)
                for h in range(8):
                    eng = nc.sync if h % 2 == 0 else nc.gpsimd
                    eng.dma_start(
                        qbds[16 * h:16 * h + 16, :].rearrange(
                            "p (s hh q) -> p s hh q", hh=8, q=128)[:, :, h, :],
                        qk[16 * h:16 * h + 16, 0:512].rearrange(
                            "p (s q) -> p s q", q=128))
                # v + eps + vaug per subtile
                vaugs = []
                vp = p1024.tile([128, 1024], F32, tag="p1024", name="vp")
                VOFF = [0, 129, 512, 641]
                for s in range(4):
                    nc.tensor.matmul(vp[:, VOFF[s]:VOFF[s] + 129],
                                     lhsT=z1[:, 128 * s:128 * (s + 1)],
                                     rhs=cs["wv_aug"][:],
                                     start=True, stop=True)
                vsb = []
                for p2 in range(2):
                    v2 = wp.tile([128, 258], BF16, tag="v_sb", name="v_sb",
                                 bufs=2)
                    nc.vector.tensor_copy(out=v2[:],
                                          in_=vp[:, 512 * p2:512 * p2 + 258])
                    vsb.append(v2)
                for s in range(4):
                    v_sb = vsb[s // 2][:, 129 * (s % 2):129 * (s % 2) + 129]
                    epsc = wp.tile([128, 1], F32, tag="epsc", name="epsc",
                                   bufs=4)
                    nc.scalar.activation(
                        epsc[:], v_sb[:, 128:129], AF.Exp,
                        bias=cs["negv"][:, 4 * g + s:4 * g + s + 1])
                    vaug = wp.tile([128, 256], BF16, tag="vaug",
                                   name="vaug", bufs=4)
                    vv = vaug[:].rearrange("p (h d) -> p h d", d=32)
                    nc.vector.tensor_scalar(
                        out=vv[:, :, 0:16],
                        in0=v_sb[:, 0:128].rearrange("p (h d) -> p h d",
                                                     d=16),
                        scalar1=epsc[:, 0:1], scalar2=None, op0=ALU.mult)
                    nc.gpsimd.tensor_copy(
                        out=vv[:, :, 16:17],
                        in_=epsc[:, 0:1].to_broadcast([128, 8, 1]))
                    vaugs.append(vaug)
                # scores -> exp -> mask -> ctx -> norm, per subtile
                ups = pacc.tile([128, 512], F32, tag="pacc", name="ups")
                cn_all = wp.tile([128, 1024], BF16, tag="cn_all",
                                 name="cn_all")
                for s in range(4):
                    scp = p1024.tile([128, 1024], F32, tag="p1024", name="scp")
                    for half in range(2):
                        nc.tensor.matmul(
                            scp[:, 512 * half:512 * (half + 1)],
                            lhsT=qk[:, 512 + 128 * s:512 + 128 * (s + 1)],
                            rhs=qbds[:, 1024 * s + 512 * half:
                                     1024 * s + 512 * (half + 1)],
                            start=True, stop=True)
                    E = wp.tile([128, 1024], BF16, tag="E", name="E")
                    nc.scalar.activation(E[:, 0:512], scp[:, 0:512], AF.Exp)
                    nc.scalar.activation(E[:, 512:1024], scp[:, 512:1024],
                                         AF.Exp)
                    Em = wp.tile([128, 1024], BF16, tag="Em", name="Em")
                    bm = (cs["bandmask16"] if ee_of(g) == 32
                          else cs["bandmask32"])
                    nc.vector.tensor_tensor(out=Em[:], in0=E[:],
                                            in1=bm[:], op=ALU.mult)
                    ctxp = pctx2p.tile([128, 1024], F32, tag="pctx2",
                                       name="ctxp")
                    for h in range(8):
                        co = 128 * (h // 4)
                        hh = 32 * (h % 4)
                        nc.tensor.matmul(
                            ctxp[hh:hh + 17, co:co + 128],
                            lhsT=vaugs[s][:].rearrange(
                                "p (h2 d) -> p h2 d", d=32)[:, h, 0:17],
                            rhs=Em[:, 128 * h:128 * (h + 1)],
                            start=True, stop=True, tile_position=(0, hh))
                    cx = wp.tile([128, 256], F32, tag="cx", name="cx")
                    nc.vector.tensor_scalar(out=cx[:], in0=ctxp[:, 0:256],
                                            scalar1=cs["epsd_col"][:, 0:1],
                                            scalar2=None, op0=ALU.add)
                    rt = wp.tile([128, 256], F32, tag="rt", name="rt")
                    nc.vector.stream_shuffle(rt[:], cx[:], SHUF16)
                    rtr = wp.tile([128, 256], F32, tag="rtr", name="rtr")
                    nc.vector.reciprocal_approx_fast(rtr[:], rt[:])
                    nc.vector.tensor_tensor(
                        out=cn_all[:, 256 * s:256 * (s + 1)], in0=cx[:],
                        in1=rtr[:], op=ALU.mult)
                cv = cn_all[:].rearrange("p (s2 hf q) -> p s2 hf q",
                                         s2=4, hf=2)
                nc.tensor.matmul(ups[:], lhsT=cs["woutA"][:],
                                 rhs=cv[:, :, 0, :], start=True, stop=False)
                nc.tensor.matmul(ups[:], lhsT=cs["woutB"][:],
                                 rhs=cv[:, :, 1, :], start=False, stop=True)
                tok2 = wp.tile([128, 512], BF16, tag="tok2", name="tok2",
                               bufs=9)
                nc.vector.scalar_tensor_tensor(
                    out=tok2[:], in0=ups[:], scalar=cs["outb_col"][:, 0:1],
                    in1=tok[:], op0=ALU.add, op1=ALU.add)
                sq2 = wp.tile([128, 512], BF16, tag="sq2", name="sq2")
                nc.vector.tensor_tensor(out=sq2[:], in0=tok2[:], in1=tok2[:],
                                        op=ALU.mult)
                st2_sb = _stats(tok2, sq2)
                return tok2, st2_sb

            def s5(g, tok2, zbc2_sb):
                """LN2 apply + FF + residual + masked pool -> xcn_all."""
                z2t = wp.tile([128, 512], BF16, tag="z2t", name="z2t")
                nc.vector.tensor_tensor(out=z2t[:], in0=tok2[:],
                                        in1=zbc2_sb[:, 0:512], op=ALU.mult)
                z2 = wp.tile([128, 512], BF16, tag="z2", name="z2")
                nc.vector.tensor_tensor(out=z2[:], in0=z2t[:],
                                        in1=zbc2_sb[:, 512:1024], op=ALU.add)
                gs = []
                for pair in range(2):
                    fp = p1024.tile([128, 1024], F32, tag="p1024", name="fp")
                    for i in range(2):
                        c4 = 2 * pair + i
                        nc.tensor.matmul(fp[:, 512 * i:512 * (i + 1)],
                                         lhsT=cs[f"wf1T_{c4}"][:], rhs=z2[:],
                                         start=True, stop=True)
                    for i in range(2):
                        c4 = 2 * pair + i
                        gt = wp.tile([128, 512], BF16, tag="gt", name="gt",
                                     bufs=4)
                        nc.scalar.activation(gt[:], fp[:, 512 * i:512 * (i + 1)],
                                             AF.Gelu,
                                             bias=cs[f"bff1_{c4}"][:, 0:1])
                        gs.append(gt)
                f2p = pacc.tile([128, 512], F32, tag="pacc", name="f2p")
                for c4 in range(4):
                    nc.tensor.matmul(f2p[:], lhsT=cs[f"wf2T_{c4}"][:],
                                     rhs=gs[c4][:], start=(c4 == 0),
                                     stop=(c4 == 3))
                tok3 = wp.tile([128, 512], BF16, tag="tok3", name="tok3")
                nc.vector.scalar_tensor_tensor(
                    out=tok3[:], in0=f2p[:], scalar=cs["bff2_col"][:, 0:1],
                    in1=tok2[:], op0=ALU.add, op1=ALU.add)
                msbc = wp.tile([128, 512], BF16, tag="msbc", name="msbc")
                nc.sync.dma_start(msbc[:],
                                  dt["msbig"][:, 512 * g:512 * (g + 1)])
                tok3m = wp.tile([128, 512], BF16, tag="tok3m", name="tok3m")
                nc.vector.tensor_tensor(out=tok3m[:], in0=tok3[:],
                                        in1=msbc[:], op=ALU.mult)
                e0, ee = edge0_of(g), ee_of(g)
                nc.vector.tensor_reduce(
                    out=xcn_all[:, e0:e0 + ee],
                    in_=tok3m[:].rearrange("p (e k) -> p e k", k=512 // ee),
                    axis=mybir.AxisListType.X, op=ALU.add)

            for g0 in range(0, nst, GRP):
                gs_ = list(range(g0, min(g0 + GRP, nst)))
                d1 = {g: s1(g) for g in gs_}
                l1 = {g: s2a(g, d1[g][1], "1") for g in gs_}
                r1 = {g: s2(g, d1[g][1], l1[g], "1") for g in gs_}
                d3 = {g: s3(g, d1[g][0], r1[g]) for g in gs_}
                l2 = {g: s2a(g, d3[g][1], "2") for g in gs_}
                r2 = {g: s2(g, d3[g][1], l2[g], "2") for g in gs_}
                for g in gs_:
                    s5(g, d3[g][0], r2[g])

            # ---------------- PHASE C: edge MLPs -------------------------
            xcn_bf = cp.tile([128, tcn], BF16, tag="xcn_bf")
            nc.vector.tensor_copy(out=xcn_bf[:], in_=xcn_all[:])

            def dense(rhs_tiles, win, bin_, act, n_ic, out_tag, w):
                outs = []
                for oc in range(2):
                    o = mp.tile([128, w], BF16, tag=f"{out_tag}{oc}",
                                name=out_tag)
                    for nh in range(max(1, w // 512)):
                        cw = min(512, w)
                        p5 = p1024.tile([128, 1024], F32, tag="p1024",
                                        name="p5")
                        for ic in range(n_ic):
                            wt = cs[win(ic, oc)]
                            r = (rhs_tiles if n_ic == 1 else rhs_tiles[ic])
                            nc.tensor.matmul(
                                p5[:, :cw], lhsT=wt[:],
                                rhs=r[:, cw * nh:cw * (nh + 1)],
                                start=(ic == 0), stop=(ic == n_ic - 1))
                        nc.scalar.activation(
                            o[:, cw * nh:cw * (nh + 1)], p5[:, :cw], act,
                            bias=cs[bin_(oc)][:, 0:1])
                    outs.append(o)
                return outs

            def _phase_c(lo, w):
                h1 = dense(xcn_bf[:, lo:lo + w], lambda ic, oc: f"wx1_{oc}",
                           lambda oc: f"bx1_{oc}", AF.Relu, 1, "h1_", w)
                h2 = dense(h1, lambda ic, oc: f"wx2_{ic}{oc}",
                           lambda oc: f"bx2_{oc}", AF.Relu, 2, "h2_", w)
                h3 = dense(h2, lambda ic, oc: f"wx3_{ic}{oc}",
                           lambda oc: f"bx3_{oc}", AF.Identity, 2, "h3_", w)
                j1 = dense(xijT_all[:, lo:lo + w],
                           lambda ic, oc: f"wxj1_{oc}",
                           lambda oc: f"bxj1_{oc}", AF.Relu, 1, "j1_", w)
                j2 = dense(j1, lambda ic, oc: f"wxj2_{ic}{oc}",
                           lambda oc: f"bxj2_{oc}", AF.Identity, 2, "j2_", w)
                zi = []
                for oc in range(2):
                    z = mp.tile([128, w], BF16, tag=f"zi{oc}", name="zi")
                    nc.vector.scalar_tensor_tensor(
                        out=z[:], in0=h3[oc][:], scalar=cs["beta_col"][:, 0:1],
                        in1=j2[oc][:], op0=ALU.mult, op1=ALU.add)
                    zi.append(z)
                zz = dense(zi, lambda ic, oc: f"wl1_{ic}{oc}",
                           lambda oc: f"bl1_{oc}", AF.Relu, 2, "zz", w)
                osb = mp.tile([1, w], BF16, tag="osb", name="osb")
                cw = min(512, w)
                for nh in range(max(1, w // 512)):
                    fo = pacc.tile([128, 512], F32, tag="pacc", name="fo")
                    nc.tensor.matmul(fo[0:1, :cw], lhsT=cs["wl2_0"][:],
                                     rhs=zz[0][:, cw * nh:cw * (nh + 1)],
                                     start=True, stop=False)
                    nc.tensor.matmul(fo[0:1, :cw], lhsT=cs["wl2_1"][:],
                                     rhs=zz[1][:, cw * nh:cw * (nh + 1)],
                                     start=False, stop=True)
                    nc.scalar.activation(osb[0:1, cw * nh:cw * (nh + 1)],
                                         fo[0:1, :cw], AF.Identity,
                                         bias=cs["bl2"][0:1, 0:1])
                osf = mp.tile([1, w], F32, tag="osf", name="osf")
                nc.vector.tensor_copy(out=osf[:], in_=osb[:])
                nc.sync.dma_start(out_dram[0:1, lo:lo + w], osf[:])

            for lo in range(0, tcn, 512):
                _phase_c(lo, min(512, tcn - lo))

    nc.finalize()
    return nc


# ---------------------------------------------------------------- host side

def _prep_shared(inp):
    f = lambda k: np.asarray(inp[k], np.float32)
    tok_w, tok_b = f("tok_w"), f("tok_b")
    g1, b1 = f("ln1_g"), f("ln1_b")
    qkv_w, qkv_b = f("qkv_w"), f("qkv_b")
    out_w, out_b = f("out_w"), f("out_b")
    g2, b2 = f("ln2_g"), f("ln2_b")
    ff1_w, ff1_b = f("ff1_w"), f("ff1_b")
    ff2_w, ff2_b = f("ff2_w"), f("ff2_b")

    d = {}
    d["w0xT"] = bfa(tok_w[:, :C].T)
    d["a1"] = bfa(tok_w[:, C:2 * C].T)
    d["a2"] = bfa(tok_w[:, 2 * C:3 * C].T)
    d["a3"] = bfa(tok_w[:, 3 * C:4 * C].T)
    d["tokb_row"] = bfa(tok_b[None, :])

    sc = 1.0 / np.sqrt(DH)
    Wq, Wk, Wv = qkv_w[:C], qkv_w[C:2 * C], qkv_w[2 * C:3 * C]
    bq, bk, bv = qkv_b[:C], qkv_b[C:2 * C], qkv_b[2 * C:3 * C]
    Wq_e = Wq * g1[None, :] * sc
    bq_e = (Wq @ b1) * sc + bq * sc
    Wk_e = Wk * g1[None, :]
    Wv_e = Wv * g1[None, :]
    bv_e = Wv @ b1 + bv
    w_ck = Wk_e.T @ bq_e
    ones = np.ones(C, np.float32)
    d["wqT"] = bfa(Wq_e.T)
    d["wkT"] = bfa(Wk_e.T)
    d["wv_aug"] = bfa(np.concatenate([Wv_e.T, w_ck[:, None]], axis=1))

    for nm, heads in (("woutA", [0, 1, 2, 3]), ("woutB", [4, 5, 6, 7])):
        w = np.zeros((128, 128), np.float32)
        for i, h in enumerate(heads):
            w[32 * i:32 * i + 16, :] = out_w[:, 16 * h:16 * h + 16].T
        d[nm] = bfa(w)
    d["outb_col"] = f32a((out_b + out_w @ bv_e)[:, None])

    for c4 in range(4):
        sl = slice(128 * c4, 128 * (c4 + 1))
        d[f"wf1T_{c4}"] = bfa((ff1_w[sl, :] * g2[None, :]).T)
        d[f"bff1_{c4}"] = f32a((ff1_w[sl, :] @ b2 + ff1_b[sl])[:, None])
        d[f"wf2T_{c4}"] = bfa(ff2_w[:, sl].T)
    d["bff2_col"] = f32a(ff2_b[:, None])

    for nm, wkey, bkey in (("wx1", "xcn_w1", "xcn_b1"),
                           ("wxj1", "xij_w1", "xij_b1")):
        W, B = f(wkey), f(bkey)
        for oc in range(2):
            sl = slice(128 * oc, 128 * (oc + 1))
            d[f"{nm}_{oc}"] = bfa(W[sl, :].T)
            d[f"b{nm[1:]}_{oc}"] = f32a(B[sl][:, None])
    for nm, wkey, bkey in (("wx2", "xcn_w2", "xcn_b2"),
                           ("wx3", "xcn_w3", "xcn_b3"),
                           ("wxj2", "xij_w2", "xij_b2"),
                           ("wl1", "lin_w1", "lin_b1")):
        W, B = f(wkey), f(bkey)
        for ic in range(2):
            for oc in range(2):
                d[f"{nm}_{ic}{oc}"] = bfa(
                    W[128 * oc:128 * (oc + 1), 128 * ic:128 * (ic + 1)].T)
        for oc in range(2):
            d[f"b{nm[1:]}_{oc}"] = f32a(B[128 * oc:128 * (oc + 1)][:, None])
    lin_w2, lin_b2 = f("lin_w2"), f("lin_b2")
    d["wl2_0"] = bfa(lin_w2[0, :128][:, None])
    d["wl2_1"] = bfa(lin_w2[0, 128:][:, None])
    d["bl2"] = f32a(lin_b2.reshape(1, 1))

    for kk in (16, 32):
        ee = 512 // kk
        Bind = np.zeros((ee, 512), np.float32)
        for e in range(ee):
            Bind[e, kk * e:kk * (e + 1)] = 1.0
        d[f"Bind{kk}"] = bfa(Bind)
        bm = np.zeros((128, 1024), np.float32)
        epb = 128 // kk          # edges per 128-token subtile
        for h in range(8):
            for e in range(epb):
                bm[kk * e:kk * (e + 1),
                   128 * h + kk * e:128 * h + kk * (e + 1)] = 1.0
        d[f"bandmask{kk}"] = bfa(bm)
    d["wmean"] = bfa(np.full((128, 1), 1.0 / 128.0))
    d["ones_rep"] = bfa(np.ones((128, 128)))
    d["eps_col"] = f32a(np.full((128, 1), 1e-5))
    d["epsd_col"] = f32a(np.full((128, 1), 1e-30))
    d["beta_col"] = f32a(np.full((128, 1),
                                 np.asarray(inp["beta"],
                                            np.float32).reshape(-1)[0]))
    return d


def _prep_core(inp, core, nst16, nst32, perm):
    sl = slice(core * TC, (core + 1) * TC)
    tar = np.asarray(inp["tar_ei"])[:, sl].astype(np.int32)[:, perm]
    cols = np.asarray(inp["cn_cols"])[sl].astype(np.int32)[perm]   # [TC, K]
    cnt = np.asarray(inp["cn_counts"])[sl].astype(np.int64)[perm]  # [TC]

    nst = nst16 + nst32
    ntok = 512 * nst
    # token -> (edge, k) maps for the two buckets
    t16 = np.arange(512 * nst16)
    e16, k16 = t16 // 16, t16 % 16
    t32 = np.arange(512 * nst32)
    e32, k32 = 32 * nst16 + t32 // 32, t32 % 32
    e_arr = np.concatenate([e16, e32])
    k_arr = np.concatenate([k16, k32])

    d = {}
    nsub = ntok // 128
    idx_flat = cols[e_arr, k_arr]
    d["idx_cn"] = np.ascontiguousarray(idx_flat.reshape(nsub, 128).T)
    ne = TC // 128
    d["idx_t0"] = np.ascontiguousarray(tar[0].reshape(ne, 128).T)
    d["idx_t1"] = np.ascontiguousarray(tar[1].reshape(ne, 128).T)

    valid = (k_arr < cnt[e_arr])
    vcol = np.ascontiguousarray(valid.reshape(nsub, 128).T)
    d["negv"] = ((~vcol) * np.float32(NEG)).astype(np.float32)

    ms = valid.astype(np.float32) / np.maximum(cnt[e_arr], 1).astype(np.float32)
    d["msbig"] = np.ascontiguousarray(
        np.broadcast_to(bfa(ms[None, :]), (128, ntok)))
    return d


def _split_counts(inp):
    """Global (nst16, nst32) and per-core permutations."""
    cnts = np.asarray(inp["cn_counts"]).reshape(NCORES, TC)
    n16 = (cnts <= 16).sum(axis=1)
    nst16 = int(min(n16) // 32)
    nst32 = (TC - 32 * nst16) // 16
    perms = [np.argsort(cnts[c], kind="stable") for c in range(NCORES)]
    return nst16, nst32, perms


_CACHE = {}
_CACHE_LOCK = threading.Lock()


def _get_nc(key):
    with _CACHE_LOCK:
        if key not in _CACHE:
            _CACHE[key] = _build_nc(*key)
        return _CACHE[key]


def run(inputs, nst=None, **spmd_kwargs):
    nst16, nst32, perms = _split_counts(inputs)
    nc = _get_nc((nst16, nst32))
    shared = _prep_shared(inputs)
    xbf = np.ascontiguousarray(
        np.asarray(inputs["x"], np.float32)).astype(ml_dtypes.bfloat16)
    in_maps = []
    for core in range(NCORES):
        m = dict(shared)
        m["xbf"] = xbf
        m.update(_prep_core(inputs, core, nst16, nst32, perms[core]))
        in_maps.append(m)
    res = run_bass_kernel_spmd(nc, in_maps, core_ids=list(range(NCORES)),
                               **spmd_kwargs)
    out = np.zeros((NCORES, TC), np.float32)
    for c in range(NCORES):
        out[c, perms[c]] = res.results[c]["out"][0]
    return out, res


def kernel(**inputs):
    out, _ = run(inputs)
    return out.reshape(T, O).astype(np.float32)
